# revision 3
# baseline (speedup 1.0000x reference)
"""CPR router kernel for Trainium2 (8 NeuronCores, data-parallel over tokens).

Math (matches the jax reference):
    h_n = l2norm(hidden_states, axis=1); p_n = l2norm(proto, axis=1)
    logits = h_n @ p_n.T                      # [T, 64] cosine sims
    w = softmax(logits, axis=1)
    routing_weights, selected_experts = top_k(w, 8)

Device strategy (per core, 2048 tokens):
    - proto is tiny: normalized + transposed on host, replicated to all cores.
    - h arrives [2048, 2048] f32. Per 128-token tile:
        DMA in -> PE transposes (128x128 blocks) -> PSUM -> copy to SBUF ->
        fp32 matmul vs protoT accumulating logits[128, 64] in PSUM.
        Row sum-of-squares via ScalarE Square+accum; inv_norm = rsqrt via
        ScalarE sqrt seed + one Newton step (ACT sqrt alone is low-precision).
        ScalarE Exp with per-partition scale=inv_norm and accumulated row sum
        gives softmax numerator+denominator in one op; VectorE reciprocal +
        tensor_scalar produce full softmax; VectorE max/max_index give the
        top-8 values and indices directly.
    - Outputs staged in SBUF as [128, 16*8] (partition-major) so the final
      DMA out is one contiguous 64KB transfer; host re-permutes.
"""

import os
from contextlib import ExitStack

import numpy as np

import concourse.bass as bass
import concourse.bacc as bacc
import concourse.mybir as mybir
import concourse.tile as tile
from concourse import masks

N_CORES = 8
T_FULL = 16384
D = 2048
E = 64
K = 8
P = 128
T_CORE = T_FULL // N_CORES  # 2048
N_TILES = T_CORE // P       # 16
N_CHUNKS = D // P           # 16

F32 = mybir.dt.float32
U32 = mybir.dt.uint32


def build_program():
    nc = bacc.Bacc(
        "TRN2", target_bir_lowering=False, debug=False, num_devices=N_CORES
    )
    h_d = nc.dram_tensor("h", [T_CORE, D], F32, kind="ExternalInput").ap()
    pt_d = nc.dram_tensor("pt", [P, N_CHUNKS * E], F32, kind="ExternalInput").ap()
    ow_d = nc.dram_tensor("out_w", [P, N_TILES * K], F32, kind="ExternalOutput").ap()
    oi_d = nc.dram_tensor("out_i", [P, N_TILES * K], U32, kind="ExternalOutput").ap()

    with tile.TileContext(nc) as tc, ExitStack() as ctx:
        singles = ctx.enter_context(tc.tile_pool(name="singles", bufs=1))
        h_pool = ctx.enter_context(tc.tile_pool(name="hin", bufs=3))
        hT_pool = ctx.enter_context(tc.tile_pool(name="hT", bufs=2))
        sq_pool = ctx.enter_context(tc.tile_pool(name="sq", bufs=2))
        small = ctx.enter_context(tc.tile_pool(name="small", bufs=3))
        psT_pool = ctx.enter_context(
            tc.tile_pool(name="psT", bufs=5, space=bass.MemorySpace.PSUM)
        )
        psL_pool = ctx.enter_context(
            tc.tile_pool(name="psL", bufs=2, space=bass.MemorySpace.PSUM)
        )

        pt_sb = singles.tile([P, N_CHUNKS * E], F32)
        nc.sync.dma_start(pt_sb[:], pt_d[:])
        ident = singles.tile([P, P], F32)
        masks.make_identity(nc, ident[:])
        w_stage = singles.tile([P, N_TILES * K], F32)
        i_stage = singles.tile([P, N_TILES * K], U32)

        for t in range(N_TILES):
            h_t = h_pool.tile([P, D], F32, tag="h_t")
            nc.sync.dma_start(h_t[:], h_d[t * P : (t + 1) * P, :])

            # Per-token sum of squares (ScalarE, fused square+row-accumulate).
            sq = sq_pool.tile([P, D], F32, tag="sq")
            ssq = small.tile([P, 1], F32, tag="ssq")
            nc.scalar.activation(
                sq[:], h_t[:], mybir.ActivationFunctionType.Square, accum_out=ssq[:]
            )

            # Transpose h tile chunk-by-chunk via PE; stage back to SBUF.
            hT = hT_pool.tile([P, D], F32, tag="hT")
            for b in range(4):
                ps = psT_pool.tile([P, 512], F32, tag="psT")
                for c4 in range(4):
                    c = b * 4 + c4
                    nc.tensor.transpose(
                        ps[:, c4 * P : (c4 + 1) * P],
                        h_t[:, c * P : (c + 1) * P],
                        ident[:],
                    )
                if b < 3:
                    nc.vector.tensor_copy(hT[:, b * 512 : (b + 1) * 512], ps[:])
                else:
                    nc.scalar.copy(hT[:, b * 512 : (b + 1) * 512], ps[:])

            # logits[tok, e] accumulated over d-chunks in PSUM (fp32 matmul).
            psl = psL_pool.tile([P, E], F32, tag="psl")
            for c in range(N_CHUNKS):
                nc.tensor.matmul(
                    psl[:],
                    lhsT=hT[:, c * P : (c + 1) * P],
                    rhs=pt_sb[:, c * E : (c + 1) * E],
                    start=(c == 0),
                    stop=(c == N_CHUNKS - 1),
                )

            # inv_norm = 1/sqrt(ssq): ACT sqrt seed, exact DVE reciprocal,
            # one Newton rsqrt step to wash out ACT sqrt table error.
            n0 = small.tile([P, 1], F32, tag="n0")
            nc.scalar.sqrt(n0[:], ssq[:])
            r0 = small.tile([P, 1], F32, tag="r0")
            nc.vector.reciprocal(r0[:], n0[:])
            t1 = small.tile([P, 1], F32, tag="t1")
            nc.vector.tensor_mul(t1[:], ssq[:], r0[:])
            t2 = small.tile([P, 1], F32, tag="t2")
            nc.vector.tensor_mul(t2[:], t1[:], r0[:])
            t3 = small.tile([P, 1], F32, tag="t3")
            nc.vector.tensor_scalar(
                t3[:], t2[:], -0.5, 1.5, op0=mybir.AluOpType.mult,
                op1=mybir.AluOpType.add,
            )
            inv_n = small.tile([P, 1], F32, tag="inv_n")
            nc.vector.tensor_mul(inv_n[:], r0[:], t3[:])

            # Softmax numerator + denominator in one ScalarE op.
            probs = small.tile([P, E], F32, tag="probs")
            den = small.tile([P, 1], F32, tag="den")
            nc.scalar.activation(
                probs[:],
                psl[:],
                mybir.ActivationFunctionType.Exp,
                scale=inv_n[:],
                accum_out=den[:],
            )
            rden = small.tile([P, 1], F32, tag="rden")
            nc.vector.reciprocal(rden[:], den[:])
            w_full = small.tile([P, E], F32, tag="w_full")
            nc.vector.tensor_scalar_mul(w_full[:], probs[:], rden[:])

            # Top-8 values (descending) + their indices.
            nc.vector.max(out=w_stage[:, t * K : (t + 1) * K], in_=w_full[:])
            nc.vector.max_index(
                out=i_stage[:, t * K : (t + 1) * K],
                in_max=w_stage[:, t * K : (t + 1) * K],
                in_values=w_full[:],
            )

        nc.sync.dma_start(ow_d[:], w_stage[:])
        nc.sync.dma_start(oi_d[:], i_stage[:])

    nc.compile()
    return nc


_CACHE = {}


def _get_program():
    if "nc" not in _CACHE:
        _CACHE["nc"] = build_program()
    return _CACHE["nc"]


def make_inputs_for_cores(hidden_states, proto):
    h = np.ascontiguousarray(np.asarray(hidden_states, dtype=np.float32))
    p = np.asarray(proto, dtype=np.float32)
    assert h.shape == (T_FULL, D) and p.shape == (E, D)
    norm = np.linalg.norm(p, axis=1, keepdims=True)
    pn = (p / np.maximum(norm, 1e-12)).astype(np.float32)
    # pt[p_, c*64+e] = pn[e, c*128+p_]  -> per-partition rows contiguous in DRAM
    pt = np.ascontiguousarray(
        pn.T.reshape(N_CHUNKS, P, E).transpose(1, 0, 2)
    ).reshape(P, N_CHUNKS * E)
    return [
        {"h": h[c * T_CORE : (c + 1) * T_CORE], "pt": pt} for c in range(N_CORES)
    ]


def unshard_outputs(results):
    w_parts, i_parts = [], []
    for c in range(N_CORES):
        ws = np.asarray(results[c]["out_w"])
        ix = np.asarray(results[c]["out_i"])
        w_parts.append(ws.reshape(P, N_TILES, K).transpose(1, 0, 2).reshape(T_CORE, K))
        i_parts.append(
            ix.reshape(P, N_TILES, K)
            .transpose(1, 0, 2)
            .reshape(T_CORE, K)
            .astype(np.int32)
        )
    return np.concatenate(w_parts, 0), np.concatenate(i_parts, 0)


def run_on_hw(hidden_states, proto, trace=False):
    from concourse.bass_utils import run_bass_kernel_spmd

    nc = _get_program()
    in_maps = make_inputs_for_cores(hidden_states, proto)
    res = run_bass_kernel_spmd(
        nc, in_maps, core_ids=list(range(N_CORES)), trace=trace
    )
    _CACHE["last_results"] = res
    return unshard_outputs(res.results)


def kernel(hidden_states, proto):
    return run_on_hw(hidden_states, proto, trace=False)


# revision 31
# speedup vs baseline: 41.8081x; 41.8081x over previous
"""CPR router kernel for Trainium2 (8 NeuronCores, data-parallel over tokens).

Math (matches the jax reference):
    h_n = l2norm(hidden_states, axis=1); p_n = l2norm(proto, axis=1)
    logits = h_n @ p_n.T                      # [T, 64] cosine sims
    w = softmax(logits, axis=1)
    routing_weights, selected_experts = top_k(w, 8)

Device strategy (per core, 2048 tokens, 16 tiles of 128 tokens):
    - proto is tiny: normalized + transposed on host, replicated to all cores.
    - h arrives [2048, 2048] f32. Per 128-token tile:
        DMA in (2 MiB batched) -> PE transposes (128x128 blocks, f32) ->
        PSUM -> copy to SBUF (VectorE/ScalarE split) -> fp32 matmul vs
        protoT accumulating logits[128, 64] in PSUM -> staged to SBUF.
        Row sum-of-squares via ScalarE Square with fused row-accumulate
        (one tile on VectorE tensor_tensor_reduce for load balance).
        inv_norm = rsqrt on VectorE only: Quake bit-trick seed + 3 Newton
        steps, batched per 4 tiles (avoids ScalarE sqrt, whose table set
        differs from exp/square/copy -- per-tile set switches cost ~2.7us
        each and dominated the first working version).
        ScalarE Exp with per-partition scale=inv_norm and accumulated row
        sum gives softmax numerator+denominator in one op; VectorE
        reciprocal + tensor_scalar produce the full softmax; VectorE
        max/max_index give the top-8 values and indices directly
        (descending, distinct indices on ties, matching jax top_k).
    - Outputs staged in SBUF as [128, 16*8] (partition-major) so DMA out is
      two contiguous 32KB transfers per tensor; host re-permutes.
"""

from contextlib import ExitStack

import numpy as np

import concourse.bass as bass
import concourse.bacc as bacc
import concourse.mybir as mybir
import concourse.tile as tile

N_CORES = 8
T_FULL = 16384
D = 2048
E = 64
K = 8
P = 128
T_CORE = T_FULL // N_CORES  # 2048
N_TILES = T_CORE // P       # 16
N_CHUNKS = D // P           # 16

F32 = mybir.dt.float32
F32R = mybir.dt.float32r
BF16 = mybir.dt.bfloat16
U32 = mybir.dt.uint32

# Transpose implementation. "f32" is the only exact mode: the BIR verifier
# requires fp32r matmul inputs to be pre-rounded to the reduced fp32r grid,
# so fp32r transposes would corrupt data.
TRANSPOSE_MODE = "f32"
# How many of the 16 tiles compute sum-of-squares on DVE (tensor_tensor_reduce)
# instead of ScalarE Square. MUST stay 0: InstTensorTensorReduce hangs the
# NEFF on this runtime ("mesh desynced" on every variant that used it).
SSQ_ON_DVE = 0
# Tiles per h DMA (2 -> 2MiB transfers, better HBM efficiency).
DMA_BATCH = 2


def build_program(transpose_mode=None, ssq_on_dve=None, dma_batch=None, reps=1):
    global TRANSPOSE_MODE, SSQ_ON_DVE, DMA_BATCH
    if transpose_mode is not None:
        TRANSPOSE_MODE = transpose_mode
    if ssq_on_dve is not None:
        SSQ_ON_DVE = ssq_on_dve
    if dma_batch is not None:
        DMA_BATCH = dma_batch
    nc = bacc.Bacc(
        "TRN2", target_bir_lowering=False, debug=False, num_devices=N_CORES
    )
    h_d = nc.dram_tensor("h", [T_CORE, D], F32, kind="ExternalInput").ap()
    pt_d = nc.dram_tensor("pt", [P, N_CHUNKS * E], F32, kind="ExternalInput").ap()
    id_dt = BF16 if TRANSPOSE_MODE == "f32r_bf16id" else F32
    id_d = nc.dram_tensor("ident", [P, P], id_dt, kind="ExternalInput").ap()
    ow_d = nc.dram_tensor("out_w", [P, N_TILES * K], F32, kind="ExternalOutput").ap()
    oi_d = nc.dram_tensor("out_i", [P, N_TILES * K], U32, kind="ExternalOutput").ap()

    with tile.TileContext(nc) as tc, ExitStack() as ctx:
        singles = ctx.enter_context(tc.tile_pool(name="singles", bufs=1))
        h_pool = ctx.enter_context(tc.tile_pool(name="hin", bufs=3))
        hT_pool = ctx.enter_context(tc.tile_pool(name="hT", bufs=2))
        sq_pool = ctx.enter_context(tc.tile_pool(name="sq", bufs=2))
        small = ctx.enter_context(tc.tile_pool(name="small", bufs=3))
        psT_pool = ctx.enter_context(
            tc.tile_pool(name="psT", bufs=6, space=bass.MemorySpace.PSUM)
        )
        psL_pool = ctx.enter_context(
            tc.tile_pool(name="psL", bufs=2, space=bass.MemorySpace.PSUM)
        )
        lsb_pool = ctx.enter_context(tc.tile_pool(name="lsb", bufs=6))

        pt_sb = singles.tile([P, N_CHUNKS * E], F32)
        nc.sync.dma_start(pt_sb[:], pt_d[:])
        ident = singles.tile([P, P], id_dt)
        nc.sync.dma_start(ident[:], id_d[:])
        w_stage = singles.tile([P, N_TILES * K], F32)
        i_stage = singles.tile([P, N_TILES * K], U32)
        # Per-token sum-of-squares and 1/sqrt staging for all 16 tiles.
        ssq_all = singles.tile([P, N_TILES], F32)
        inv_all = singles.tile([P, N_TILES], F32)
        rs_t1 = singles.tile([P, N_TILES], F32)
        rs_t2 = singles.tile([P, N_TILES], F32)

        def rsqrt_group(g, gw):
            """inv_all[:, g:g+gw] = rsqrt(ssq_all[:, g:g+gw]) on DVE only:
            Quake bit-trick seed + 3 Newton steps (no ACT table switch)."""
            xs = ssq_all[:, g : g + gw]
            ys = inv_all[:, g : g + gw]
            t1 = rs_t1[:, g : g + gw]
            t2 = rs_t2[:, g : g + gw]
            xu = xs.bitcast(U32)
            yu = ys.bitcast(U32)
            # yu = NOT(xu >> 1); then yu -= (NOT 0) - magic  ->  magic - (xu>>1)
            nc.vector.tensor_scalar(
                yu, xu, 1, 0xFFFFFFFF,
                op0=mybir.AluOpType.logical_shift_right,
                op1=mybir.AluOpType.bitwise_xor,
            )
            nc.vector.tensor_scalar(
                yu, yu, 0xFFFFFFFF - 0x5F3759DF, None,
                op0=mybir.AluOpType.subtract,
            )
            for _ in range(3):
                nc.vector.tensor_mul(t1, xs, ys)
                nc.vector.tensor_mul(t2, t1, ys)
                nc.vector.tensor_scalar(
                    t2, t2, -0.5, 1.5,
                    op0=mybir.AluOpType.mult, op1=mybir.AluOpType.add,
                )
                nc.vector.tensor_mul(ys, ys, t2)

        GRP = 4
        # DRAM view [128 part, 16 tile, 2048 d] so one DMA can fetch 2 tiles.
        h_v = h_d.rearrange("(a p) d -> p a d", p=P)
        h2_tiles = {}

        def phase_a(t):
            """DMA in, sum-of-squares, transpose, logits matmul -> SBUF tile."""
            nb = DMA_BATCH
            if t % nb == 0:
                h2 = h_pool.tile([P, nb, D], F32, tag="h_t")
                nc.sync.dma_start(h2[:, :, :], h_v[:, t : t + nb, :])
                h2_tiles[t] = h2
            h_t = h2_tiles[t - (t % nb)][:, t % nb, :]

            # Per-token sum of squares (fused square+row-accumulate).
            sq = sq_pool.tile([P, D], F32, tag="sq")
            ssq_dve_tiles = {
                (i * N_TILES) // SSQ_ON_DVE for i in range(SSQ_ON_DVE)
            } if SSQ_ON_DVE else set()
            if t in ssq_dve_tiles:
                nc.vector.tensor_tensor_reduce(
                    out=sq[:],
                    in0=h_t[:],
                    in1=h_t[:],
                    scale=1.0,
                    scalar=0.0,
                    op0=mybir.AluOpType.mult,
                    op1=mybir.AluOpType.add,
                    accum_out=ssq_all[:, t : t + 1],
                )
            else:
                nc.scalar.activation(
                    sq[:],
                    h_t[:],
                    mybir.ActivationFunctionType.Square,
                    accum_out=ssq_all[:, t : t + 1],
                )

            # Transpose h tile chunk-by-chunk via PE; stage back to SBUF.
            hT = hT_pool.tile([P, D], F32, tag="hT")
            for b in range(4):
                ps = psT_pool.tile([P, 512], F32, tag="psT")
                for c4 in range(4):
                    c = b * 4 + c4
                    src = h_t[:, c * P : (c + 1) * P]
                    dst = ps[:, c4 * P : (c4 + 1) * P]
                    if TRANSPOSE_MODE == "f32":
                        nc.tensor.transpose(dst, src, ident[:])
                    else:
                        rhs_id = (
                            ident[:].bitcast(F32R)
                            if TRANSPOSE_MODE == "f32r"
                            else ident[:]
                        )
                        nc.tensor.matmul(
                            dst.bitcast(F32R),
                            lhsT=src.bitcast(F32R),
                            rhs=rhs_id,
                            is_transpose=True,
                        )
                if b < 3:
                    nc.vector.tensor_copy(hT[:, b * 512 : (b + 1) * 512], ps[:])
                else:
                    nc.scalar.copy(hT[:, b * 512 : (b + 1) * 512], ps[:])

            # logits[tok, e] accumulated over d-chunks in PSUM (fp32 matmul),
            # then staged to SBUF so the PSUM bank frees immediately.
            psl = psL_pool.tile([P, E], F32, tag="psl")
            for c in range(N_CHUNKS):
                nc.tensor.matmul(
                    psl[:],
                    lhsT=hT[:, c * P : (c + 1) * P],
                    rhs=pt_sb[:, c * E : (c + 1) * E],
                    start=(c == 0),
                    stop=(c == N_CHUNKS - 1),
                )
            lsb = lsb_pool.tile([P, E], F32, tag="lsb")
            nc.vector.tensor_copy(lsb[:], psl[:])
            return lsb

        def phase_b(t, lsb):
            """Softmax (fused exp+rowsum) and top-8 selection."""
            probs = small.tile([P, E], F32, tag="probs")
            den = small.tile([P, 1], F32, tag="den")
            nc.scalar.activation(
                probs[:],
                lsb[:],
                mybir.ActivationFunctionType.Exp,
                scale=inv_all[:, t : t + 1],
                accum_out=den[:],
            )
            rden = small.tile([P, 1], F32, tag="rden")
            nc.vector.reciprocal(rden[:], den[:])
            w_full = small.tile([P, E], F32, tag="w_full")
            nc.vector.tensor_scalar_mul(w_full[:], probs[:], rden[:])

            # Top-8 values (descending) + their indices.
            nc.vector.max(out=w_stage[:, t * K : (t + 1) * K], in_=w_full[:])
            nc.vector.max_index(
                out=i_stage[:, t * K : (t + 1) * K],
                in_max=w_stage[:, t * K : (t + 1) * K],
                in_values=w_full[:],
            )

        for _rep in range(reps):
            for g in range(0, N_TILES, GRP):
                psls = [phase_a(t) for t in range(g, g + GRP)]
                rsqrt_group(g, GRP)
                for i, t in enumerate(range(g, g + GRP)):
                    phase_b(t, psls[i])
                # Stream finished halves out early to shorten the tail.
                if g + GRP == N_TILES // 2:
                    half = N_TILES // 2 * K
                    nc.sync.dma_start(ow_d[:, :half], w_stage[:, :half])
                    nc.sync.dma_start(oi_d[:, :half], i_stage[:, :half])

        half = N_TILES // 2 * K
        nc.sync.dma_start(ow_d[:, half:], w_stage[:, half:])
        nc.sync.dma_start(oi_d[:, half:], i_stage[:, half:])

    nc.compile()
    return nc


_CACHE = {}


def _get_program():
    if "nc" not in _CACHE:
        _CACHE["nc"] = build_program()
    return _CACHE["nc"]


def make_inputs_for_cores(hidden_states, proto):
    h = np.ascontiguousarray(np.asarray(hidden_states, dtype=np.float32))
    p = np.asarray(proto, dtype=np.float32)
    assert h.shape == (T_FULL, D) and p.shape == (E, D)
    norm = np.linalg.norm(p, axis=1, keepdims=True)
    pn = (p / np.maximum(norm, 1e-12)).astype(np.float32)
    # pt[p_, c*64+e] = pn[e, c*128+p_]  -> per-partition rows contiguous in DRAM
    pt = np.ascontiguousarray(
        pn.T.reshape(N_CHUNKS, P, E).transpose(1, 0, 2)
    ).reshape(P, N_CHUNKS * E)
    id_np = np.eye(P, dtype=np.float32)
    if TRANSPOSE_MODE == "f32r_bf16id":
        import ml_dtypes

        id_np = id_np.astype(ml_dtypes.bfloat16)
    return [
        {"h": h[c * T_CORE : (c + 1) * T_CORE], "pt": pt, "ident": id_np}
        for c in range(N_CORES)
    ]


def unshard_outputs(results):
    w_parts, i_parts = [], []
    for c in range(N_CORES):
        ws = np.asarray(results[c]["out_w"])
        ix = np.asarray(results[c]["out_i"])
        w_parts.append(ws.reshape(P, N_TILES, K).transpose(1, 0, 2).reshape(T_CORE, K))
        i_parts.append(
            ix.reshape(P, N_TILES, K)
            .transpose(1, 0, 2)
            .reshape(T_CORE, K)
            .astype(np.int32)
        )
    return np.concatenate(w_parts, 0), np.concatenate(i_parts, 0)


def run_on_hw(hidden_states, proto, trace=False):
    from concourse.bass_utils import run_bass_kernel_spmd

    nc = _get_program()
    in_maps = make_inputs_for_cores(hidden_states, proto)
    res = run_bass_kernel_spmd(
        nc, in_maps, core_ids=list(range(N_CORES)), trace=trace
    )
    _CACHE["last_results"] = res
    return unshard_outputs(res.results)


def kernel(hidden_states, proto):
    return run_on_hw(hidden_states, proto, trace=False)
